# revision 24
# baseline (speedup 1.0000x reference)
"""GCN layer kernel for Trainium2, 8 NeuronCores (SPMD).

Math (see reference):
    deg = scatter_add(ones, row); deg = max(deg, 1)
    norm_e = rsqrt(deg[row_e]) * rsqrt(deg[col_e])
    agg[row_e] += x[col_e] * norm_e
    out = agg @ W.T + b

Device strategy (v2):
  - Shard DESTINATION nodes across 8 cores (12500 each) -> no collective;
    each core computes its own slice of the output.
  - Edges sorted by (dest supergroup, src chunk, dest window, src), padded
    per (sg, chunk, win) group to 128-edge tiles.
  - x is gathered in bf16 (256B rows; halves HBM gather traffic vs f32)
    via gpsimd dma_gather, one call per (supergroup, chunk) split into
    <=1024-descriptor pieces (the SWDGE ring holds ~1024 descriptors; a
    bigger call stalls the Pool engine on ring space).
  - Scatter-add on the TensorEngine per 128-dest window:
        aggT[f, d] += msgs[e, f]^T @ onehot[e, d]
    with onehot built in bf16 by one fused DVE tensor_scalar
    (is_equal, mult) per 128-edge tile.  bf16 onehot hits the DVE 2x
    mode (~2.2x faster than the f32r onehot) and the WD=128 window
    halves per-tile DVE+PE work vs WD=256.
  - Epilogue per window: PSUM->SBUF copy (scalar engine, bf16), dense
    128x128 matmul with W^T, bias added via a K=1 PE matmul
    (ones^T @ b) accumulated into the same PSUM tile; no DVE work.

Pitfalls baked in: single_packet=True wedges the device (never use);
dyn-repeat hardware loops wedge the device; keep gather calls <=1024
descriptors.
"""

import numpy as np
from contextlib import ExitStack
import ml_dtypes

N_NODES = 100000
N_EDGES = 1600000
D = 128
NCORES = 8
NLOC = N_NODES // NCORES          # 12500 real dests per core
P = 128
CHUNK = 32768                     # x chunk rows (int16 index range)
NCHUNK = (N_NODES + CHUNK - 1) // CHUNK  # 4

WD = 128                          # dest window
SGW = 8                           # windows per supergroup (gather granularity)

bf16 = ml_dtypes.bfloat16


def _host_prep(x, edge_index, W, b, wd=None, sgw=None, colsort=True,
               padneg=False):
    global WD, SGW
    if wd is not None:
        WD = wd
    if sgw is not None:
        SGW = sgw
    NWIN = (NLOC + WD - 1) // WD
    NPAD = NWIN * WD
    NSG = (NWIN + SGW - 1) // SGW

    row = np.asarray(edge_index[0], dtype=np.int64)
    col = np.asarray(edge_index[1], dtype=np.int64)

    deg = np.bincount(row, minlength=N_NODES).astype(np.float32)
    deg = np.maximum(deg, 1.0)
    rs = 1.0 / np.sqrt(deg)
    norm = (rs[row] * rs[col]).astype(np.float32)

    core = row // NLOC
    local = row - core * NLOC
    win = local // WD
    sg = win // SGW
    chunk = col >> 15
    key = (((core * NSG + sg) * NCHUNK + chunk) * NWIN + win).astype(np.int64)
    if colsort:
        # secondary sort by DESTINATION within each group: a 128-edge tile
        # then spans a narrow dest range [lo,hi), so the onehot build and
        # scatter matmul only touch span columns instead of the full window
        order = np.argsort(key * N_NODES + local, kind="stable")
    else:
        order = np.argsort(key, kind="stable")
    key_s = key[order]
    col_s = col[order]
    local_s = local[order]
    norm_s = norm[order]

    nbins = NCORES * NSG * NCHUNK * NWIN
    counts = np.bincount(key_s, minlength=nbins).reshape(NCORES, NSG, NCHUNK, NWIN)
    starts = np.zeros(nbins + 1, dtype=np.int64)
    np.cumsum(counts.reshape(-1), out=starts[1:])

    # tiles per (sg, chunk, win): max over cores (same schedule on all cores)
    Tt = np.ceil(counts.max(axis=0) / P).astype(np.int64)  # [NSG, NCHUNK, NWIN]
    total_tiles = int(Tt.sum())
    total_edges_padded = total_tiles * P

    # --- static schedule (identical across cores) ---
    # gather pieces are packed to <= PIECE_T tiles (SWDGE ring ~1024 descs)
    # and aligned to end at (sg,c,w) group boundaries so each core's trailing
    # padding can be marked idx=-1 (the gather ucode skips tail negatives).
    PIECE_T = 8
    sgs = []
    tail_ok = np.zeros((NSG, NCHUNK, NWIN), dtype=bool)
    tile_col = 0
    max_sg_tiles = 0
    for s in range(NSG):
        wlo, whi = s * SGW, min((s + 1) * SGW, NWIN)
        calls = []
        wtiles = {w: [] for w in range(wlo, whi)}
        soff = 0
        for c in range(NCHUNK):
            tn = int(Tt[s, c, wlo:whi].sum())
            if tn:
                calls.append((c, tile_col + soff, soff, tn))
            o = 0
            for w in range(wlo, whi):
                tw = int(Tt[s, c, w])
                for j in range(tw):
                    wtiles[w].append((tile_col + soff + o + j, soff + o + j))
                o += tw
            assert o == tn
            soff += tn
        ntiles = soff
        max_sg_tiles = max(max_sg_tiles, ntiles)
        wins = [(w, wtiles[w]) for w in range(wlo, whi)]
        sgs.append(dict(calls=calls, wins=wins, ntiles=ntiles))
        tile_col += ntiles
    assert tile_col == total_tiles

    AUXW = 2 * WD + D
    CW = 2 * total_tiles

    iota = np.tile(np.arange(WD, dtype=np.float32), (P, 1)).astype(bf16)
    ones = np.ones((P, WD), dtype=bf16)
    brep = np.tile(np.asarray(b, np.float32).reshape(1, D), (P, 1)).astype(bf16)
    aux = np.concatenate([iota, ones, brep], axis=1)
    WT16 = np.ascontiguousarray(np.asarray(W, np.float32).T).astype(bf16)
    x16 = np.ascontiguousarray(np.asarray(x, np.float32)).astype(bf16)

    idx_cols = total_edges_padded // 16
    in_maps = []
    all_dl, all_nm = [], []
    for k in range(NCORES):
        dl_flat = np.zeros(total_edges_padded, np.float32)
        nm_flat = np.zeros(total_edges_padded, np.float32)
        # default pad idx 0 (gathers row 0 harmlessly); piece-tail padding is
        # set to -1 below when padneg (ucode skips trailing negatives only)
        ix_flat = np.zeros(total_edges_padded, np.int16)
        off = 0
        for s in range(NSG):
            wlo, whi = s * SGW, min((s + 1) * SGW, NWIN)
            for c in range(NCHUNK):
                for w in range(wlo, whi):
                    t = int(Tt[s, c, w])
                    if t == 0:
                        continue
                    g = ((k * NSG + s) * NCHUNK + c) * NWIN + w
                    sstart, send = starts[g], starts[g + 1]
                    n = int(send - sstart)
                    sl = slice(off, off + n)
                    ix_flat[sl] = (col_s[sstart:send] - (c << 15)).astype(np.int16)
                    dl_flat[sl] = (local_s[sstart:send] - w * WD).astype(np.float32)
                    nm_flat[sl] = norm_s[sstart:send]
                    # pad edges: src 0 (chunk-local), norm 0, dloc 0; when the
                    # group ends a gather piece, mark its tail -1 so the ucode
                    # skips those descriptors (per-core adaptive)
                    if padneg and tail_ok[s, c, w]:
                        ix_flat[off + n:off + t * P] = -1
                    off += t * P
        assert off == total_edges_padded

        # gathered row i of a call lands at msgs[p = i%128, tile i//128]:
        # per-tile column layout for dloc/nrm = [P, total_tiles]
        dloc2 = dl_flat.reshape(total_tiles, P).T
        nrm2 = nm_flat.reshape(total_tiles, P).T
        consts = np.concatenate([dloc2, nrm2], axis=1).astype(np.float32)

        # idx table: wrapped in 16 partitions, replicated to 128 (8 q7 cores)
        idx16 = np.ascontiguousarray(ix_flat.reshape(idx_cols, 16).T)
        idx128 = np.tile(idx16, (8, 1))

        in_maps.append({
            "x": x16,
            "idxs": idx128,
            "consts": consts,
            "aux": aux,
            "wt": WT16,
        })
        all_dl.append(dl_flat)
        all_nm.append(nm_flat)

    # per-tile real-dest span [lo,hi) across all cores (padding nrm=0 excluded)
    dls = np.stack(all_dl).reshape(NCORES, total_tiles, P)
    nms = np.stack(all_nm).reshape(NCORES, total_tiles, P)
    spans = []
    for j in range(total_tiles):
        m = nms[:, j, :] > 0
        if m.any():
            d = dls[:, j, :][m]
            spans.append((int(d.min()), int(d.max()) + 1))
        else:
            spans.append((0, 1))

    plan = dict(sgs=sgs, max_sg_tiles=max_sg_tiles, total_tiles=total_tiles,
                WD=WD, SGW=SGW, NWIN=NWIN, NPAD=NPAD, spans=spans)
    layout = dict(idx_cols=idx_cols, AUXW=AUXW)
    return in_maps, plan, CW, layout


def _build_nc(plan, CW, layout, repeat=1, nqueues=4, ohmode="span", sp=False,
              parts=("gather", "onehot", "matmul", "epilogue"),
              gbufs=2, ohbufs=8, psbufs=4, ohdt="bf16", max_idx=1024,
              scratch=49152, pool_oh=0, zinit=True):
    from concourse import bacc, mybir
    import concourse.tile as tile

    f32 = mybir.dt.float32
    bf = mybir.dt.bfloat16
    i16 = mybir.dt.int16

    WD_ = plan["WD"]
    NPAD = plan["NPAD"]
    NT = plan["total_tiles"]
    idx_cols = layout["idx_cols"]
    AUXW = layout["AUXW"]
    ohdtype = bf if ohdt == "bf16" else mybir.dt.float32r

    nc = bacc.Bacc("TRN2", num_swdge_queues=nqueues,
                   dynamic_dma_scratch_size=scratch)
    x_ext = nc.declare_dram_parameter("x", [N_NODES, D], bf, isOutput=False)
    idx_ext = nc.declare_dram_parameter("idxs", [P, idx_cols], i16, isOutput=False)
    c_ext = nc.declare_dram_parameter("consts", [P, CW], f32, isOutput=False)
    aux_ext = nc.declare_dram_parameter("aux", [P, AUXW], bf, isOutput=False)
    wt_ext = nc.declare_dram_parameter("wt", [D, D], bf, isOutput=False)
    out_ext = nc.declare_dram_parameter("out", [NPAD, D], f32, isOutput=True)

    with tile.TileContext(nc) as tc:
        with ExitStack() as ctx:
            const = ctx.enter_context(tc.tile_pool(name="const", bufs=1))
            gat = ctx.enter_context(tc.tile_pool(name="gat", bufs=gbufs))
            oh_pool = ctx.enter_context(tc.tile_pool(name="oh", bufs=ohbufs))
            ep = ctx.enter_context(tc.tile_pool(name="ep", bufs=3))
            psum = ctx.enter_context(tc.tile_pool(name="psum", bufs=psbufs, space="PSUM"))
            psum_o = ctx.enter_context(tc.tile_pool(name="psum_o", bufs=2, space="PSUM"))
            psum_c = ctx.enter_context(tc.tile_pool(name="psum_c", bufs=1, space="PSUM"))

            idx_sb = const.tile([P, idx_cols], i16)
            nc.sync.dma_start(idx_sb[:], idx_ext[:])
            c_sb = const.tile([P, CW], f32)
            nc.sync.dma_start(c_sb[:], c_ext[:])
            aux_sb = const.tile([P, AUXW], bf)
            nc.sync.dma_start(aux_sb[:], aux_ext[:])
            wt_sb = const.tile([D, D], bf)
            nc.sync.dma_start(wt_sb[:], wt_ext[:])

            iota_ap = aux_sb[:, 0:WD_]
            ones_row = aux_sb[0:1, WD_:2 * WD_]
            b_row = aux_sb[0:1, 2 * WD_:2 * WD_ + D]
            if ohmode == "psum":
                # iota read stream moved to PSUM: relieves SBUF port
                # contention (DVE oh writes + PE msgs/oh reads + DMA msgs
                # writes all target SBUF)
                iota_ps = psum_c.tile([P, WD_], f32, space="PSUM")
                nc.vector.tensor_copy(iota_ps[:], aux_sb[:, 0:WD_])
                iota_ap = iota_ps[:]

            if zinit and "gather" in parts and "fakegather" not in parts:
                # zero the rotating msgs buffers once so descriptor-skipped
                # padding slots (padneg) never read as NaN garbage
                for _z in range(gbufs):
                    zt = gat.tile([P, plan["max_sg_tiles"] * D], bf, tag="msgs")
                    nc.vector.memset(zt[:], 0)

            fake_msgs = None
            if "fakegather" in parts:
                fake_msgs = const.tile([P, plan["max_sg_tiles"] * D], bf)
                nc.sync.dma_start(
                    fake_msgs[:],
                    x_ext[0:P * plan["max_sg_tiles"], :].rearrange("(p t) d -> p (t d)", p=P))

            def _epilogue(w, aggT_ps):
                aggT_sb = ep.tile([P, WD_], bf, tag="aggT")
                nc.scalar.copy(aggT_sb[:], aggT_ps[:])
                for h in range((WD_ + P - 1) // P):
                    hp = min(P, WD_ - h * P)
                    out_ps = psum_o.tile([hp, D], f32, space="PSUM")
                    nc.tensor.matmul(out=out_ps[:], lhsT=aggT_sb[:, h * P:h * P + hp],
                                     rhs=wt_sb[:], start=True, stop=False)
                    nc.tensor.matmul(out=out_ps[:], lhsT=ones_row[:, :hp], rhs=b_row,
                                     start=False, stop=True)
                    out_sb = ep.tile([hp, D], f32, tag="out")
                    nc.scalar.copy(out_sb[:], out_ps[:])
                    nc.sync.dma_start(out_ext[w * WD_ + h * P:w * WD_ + h * P + hp, :], out_sb[:])

            qn = 0
            for _rep in range(repeat):
                pending = None  # (w, aggT_ps) whose epilogue is deferred one window
                for info in plan["sgs"]:
                    if fake_msgs is not None:
                        msgs = fake_msgs
                    else:
                        msgs = gat.tile([P, plan["max_sg_tiles"] * D], bf, tag="msgs")
                        if "gather" in parts:
                            for (c, icol_t, toff, tn) in info["calls"]:
                                tstep = tn if max_idx == 0 else max(1, max_idx // P)
                                for t0 in range(0, tn, tstep):
                                    tcur = min(tstep, tn - t0)
                                    nc.gpsimd.dma_gather(
                                        out_ap=msgs[:, (toff + t0) * D:(toff + t0 + tcur) * D]
                                        .rearrange("p (c d) -> p c d", d=D),
                                        in_ap=x_ext[c * CHUNK:min((c + 1) * CHUNK, N_NODES), :],
                                        idxs_ap=idx_sb[:, (icol_t + t0) * (P // 16):(icol_t + t0 + tcur) * (P // 16)],
                                        num_idxs=tcur * P,
                                        num_idxs_reg=tcur * P,
                                        elem_size=D,
                                        single_packet=sp,
                                        queue_num=(qn % nqueues),
                                    )
                                    qn += 1
                    for (w, tlist) in info["wins"]:
                        ntl = len(tlist)
                        if ntl == 0:
                            continue
                        aggT_ps = psum.tile([P, WD_], f32, space="PSUM")
                        for k, (gcol, soff) in enumerate(tlist):
                            # tiles are dest-sorted: tile k>0 touches only
                            # [lo,hi) of the window; tile 0 runs full width
                            # with start=True to initialize the whole PSUM
                            if ohmode == "span" and k > 0:
                                lo, hi = plan["spans"][gcol]
                            else:
                                lo, hi = 0, WD_
                            span = hi - lo
                            oh = oh_pool.tile([P, WD_], ohdtype)
                            if "onehot" in parts:
                                nc.vector.tensor_scalar(
                                    out=oh[:, :span],
                                    in0=aux_sb[:, lo:hi],
                                    scalar1=c_sb[:, gcol:gcol + 1],
                                    scalar2=c_sb[:, NT + gcol:NT + gcol + 1],
                                    op0=mybir.AluOpType.is_equal,
                                    op1=mybir.AluOpType.mult,
                                )
                            if "matmul" in parts:
                                nc.tensor.matmul(
                                    out=aggT_ps[:, lo:hi],
                                    lhsT=msgs[:, soff * D:(soff + 1) * D],
                                    rhs=oh[:, :span],
                                    start=(k == 0),
                                    stop=(k == ntl - 1),
                                    skip_group_check=(ohmode == "span"),
                                )
                        if "epilogue" not in parts or "matmul" not in parts:
                            continue
                        # defer this window's epilogue until the NEXT window's
                        # scatter matmuls are emitted, so the PE stream isn't
                        # stalled on the Act PSUM->SBUF copy between windows
                        if pending is not None:
                            _epilogue(*pending)
                        pending = (w, aggT_ps)
                if pending is not None:
                    _epilogue(*pending)
                    pending = None

    nc.compile()
    return nc


def run(x, edge_index, W, b, trace=False, **cfg):
    """Build + run on 8 cores. Returns (out, results)."""
    from concourse.bass_utils import run_bass_kernel_spmd

    build_keys = ("nqueues", "ohmode", "sp", "gbufs", "ohbufs", "psbufs",
                  "ohdt", "max_idx", "parts", "scratch", "pool_oh", "zinit")
    bcfg = {k: v for k, v in cfg.items() if k in build_keys}
    pcfg = {k: v for k, v in cfg.items() if k in ("wd", "sgw", "colsort", "padneg")}
    in_maps, plan, CW, layout = _host_prep(x, edge_index, W, b, **pcfg)
    nc = _build_nc(plan, CW, layout, **bcfg)
    res = run_bass_kernel_spmd(nc, in_maps, list(range(NCORES)), trace=trace)
    parts = [res.results[k]["out"][:NLOC] for k in range(NCORES)]
    out = np.concatenate(parts, axis=0).astype(np.float32)
    return out, res


def kernel(x, edge_index, W, b):
    out, _ = run(x, edge_index, W, b)
    return out


# ---------------------------------------------------------------------------
# benchmarking: time repeat=R vs repeat=1 NEFFs with device-resident inputs;
# the delta cancels transfers/dispatch and yields per-iteration HW time.
# NOTE: never keep two compiled executables alive and alternate them — that
# wedges the axon mesh.  Build+time one NEFF fully before the next.
# ---------------------------------------------------------------------------

def _make_callable(nc, in_maps):
    import jax
    import numpy as _np
    from jax.sharding import Mesh, PartitionSpec, NamedSharding
    from jax.experimental.shard_map import shard_map
    from concourse import mybir
    from concourse.bass2jax import (
        _bass_exec_p, install_neuronx_cc_hook, partition_id_tensor,
    )

    install_neuronx_cc_hook()
    n_cores = len(in_maps)
    in_names, out_names, out_avals, zero_outs = [], [], [], []
    for alloc in nc.m.functions[0].allocations:
        if not isinstance(alloc, mybir.MemoryLocationSet):
            continue
        name = alloc.memorylocations[0].name
        if alloc.kind == "ExternalInput":
            if nc.partition_id_tensor is None or name != nc.partition_id_tensor.name:
                in_names.append(name)
        elif alloc.kind == "ExternalOutput":
            out_names.append(name)
            shape = tuple(alloc.tensor_shape)
            dtype = mybir.dt.np(alloc.dtype)
            out_avals.append(jax.core.ShapedArray(shape, dtype))
            zero_outs.append(_np.zeros(shape, dtype))
    n_params = len(in_names)
    all_in_names = in_names + out_names
    if nc.partition_id_tensor is not None:
        all_in_names = all_in_names + [nc.partition_id_tensor.name]

    def _body(*args):
        operands = list(args)
        if nc.partition_id_tensor is not None:
            operands.append(partition_id_tensor())
        outs = _bass_exec_p.bind(
            *operands,
            out_avals=tuple(out_avals),
            in_names=tuple(all_in_names),
            out_names=tuple(out_names),
            lowering_input_output_aliases=(),
            sim_require_finite=True,
            sim_require_nnan=True,
            nc=nc,
        )
        return tuple(outs)

    devices = jax.devices()[:n_cores]
    mesh = Mesh(_np.asarray(devices), ("core",))
    spec = PartitionSpec("core")
    in_specs = (spec,) * (n_params + len(out_names))
    out_specs = (spec,) * len(out_names)
    fn = jax.jit(shard_map(_body, mesh=mesh, in_specs=in_specs,
                           out_specs=out_specs, check_rep=False),
                 keep_unused=True)
    sharding = NamedSharding(mesh, spec)
    dev_in = [
        jax.device_put(
            _np.concatenate([_np.asarray(in_maps[c][nm]) for c in range(n_cores)], axis=0),
            sharding)
        for nm in in_names
    ]
    dev_zero = [
        jax.device_put(_np.zeros((n_cores * z.shape[0], *z.shape[1:]), z.dtype), sharding)
        for z in zero_outs
    ]
    return fn, dev_in, dev_zero


def bench(x, edge_index, W, b, big_repeat=5, iters=6):
    import time
    import jax

    in_maps, plan, CW, layout = _host_prep(x, edge_index, W, b)
    times = {}
    for R in (1, big_repeat):
        nc = _build_nc(plan, CW, layout, repeat=R)
        fn, dev_in, dev_zero = _make_callable(nc, in_maps)
        outs = fn(*dev_in, *dev_zero)  # compile + warm
        jax.block_until_ready(outs)
        best = float("inf")
        for _ in range(iters):
            t0 = time.perf_counter()
            outs = fn(*dev_in, *dev_zero)
            jax.block_until_ready(outs)
            best = min(best, time.perf_counter() - t0)
        times[R] = best
        del fn, dev_in, dev_zero
        print(f"repeat={R}: best wall {best*1e3:.3f} ms")
    per_iter_ns = (times[big_repeat] - times[1]) / (big_repeat - 1) * 1e9
    return per_iter_ns, times
